# revision 18
# baseline (speedup 1.0000x reference)
"""Trainium2 Bass kernel for nn_LorentzRankingLoss.

Contract: kernel(**inputs) takes FULL unsharded numpy inputs
(voxel_emb [2,64,96,96,96] f32, labels [2,96,96,96] int, label_emb [128,64] f32)
and returns the FULL output (scalar f32 loss), distributing work over 8
NeuronCores internally.

Algorithm notes
---------------
The reference samples NUM_SAMPLES=64 voxels per class (128 classes) by a
stable argsort of key = label*2 + pri where pri = uniform(jax key 42) is an
*input-independent* constant.  Likewise the random negative-class choices
per sampled slot are input-independent.  So:

* pri, the candidate set {n : pri[n] < T}, and the negative-selection masks
  are compile-time constants (computed once, host side).
* The top-64-per-class selection only needs the labels of the ~17.6k
  candidate voxels (a class's 64 smallest priorities all lie below T=0.01
  with astronomically high probability; verified at runtime with an exact
  full fallback).
* The 8192 selected anchor rows are gathered on host (strided rows of
  voxel_emb) and the 8 NeuronCores compute the Lorentz distance matrix
  (8192x128 via TensorE with a +time extended contraction), acosh, the
  margin/negative-masked triplet terms and the full reduction; each core
  handles 1024 slots and emits one partial sum.
"""

import numpy as np

# ---- problem constants (hardcoded per spec) ----
NUM_SAMPLES = 64
NUM_NEG = 8
C = 128
MARGIN = 0.1
CURV = 1.0
EPS = 1e-7
B, D, H, W, Z = 2, 64, 96, 96, 96
HWZ = H * W * Z
N = B * HWZ                      # 1_769_472
KMAX = C * NUM_SAMPLES           # 8192
NCORES = 8
KPC = KMAX // NCORES             # 1024 slots per core
NCHUNK = KPC // 128              # 8 chunks of 128 anchors
CAND_T = np.float32(0.01)        # candidate priority threshold
CAND_T_SAFE = np.float32(0.01 - 1e-4)

_consts = None                   # lazy: (pri, cand_idx, negmask, posmask)
_nc = None                       # lazy: compiled bass program


# --------------------------------------------------------------------------
# host-side constants (input independent)
# --------------------------------------------------------------------------
def _build_constants():
    global _consts
    if _consts is not None:
        return _consts
    import jax
    import jax.numpy as jnp

    cpu = jax.devices("cpu")[0]
    with jax.default_device(cpu):
        key = jax.random.key(42)
        k_pri, k_neg = jax.random.split(key)
        pri = np.asarray(jax.random.uniform(k_pri, (N,), dtype=jnp.float32))
        neg_scores = np.asarray(
            jax.random.uniform(k_neg, (KMAX, C), dtype=jnp.float32)
        )

    cand_idx = np.nonzero(pri < CAND_T)[0].astype(np.int32)

    sampled_classes = (np.arange(KMAX) // NUM_SAMPLES).astype(np.int32)
    nmask_bool = np.arange(C)[None, :] != sampled_classes[:, None]
    scores = np.where(nmask_bool, neg_scores, -1.0).astype(np.float32)
    # jax.lax.top_k: descending, ties -> lower index first == stable argsort
    neg_idx = np.argsort(-scores, axis=1, kind="stable")[:, :NUM_NEG]
    negmask = np.zeros((KMAX, C), np.float32)
    np.put_along_axis(negmask, neg_idx, 1.0, axis=1)

    # per-core class permutation: the 16 classes whose slots a core owns
    # come first, so positive-distance extraction is SPMD-uniform
    perms = []
    negmaskT_cores = []
    for i in range(NCORES):
        own = np.arange(16 * i, 16 * (i + 1))
        rest = np.concatenate([np.arange(0, 16 * i), np.arange(16 * (i + 1), C)])
        perm = np.concatenate([own, rest]).astype(np.int64)
        perms.append(perm)
        sl = slice(i * KPC, (i + 1) * KPC)
        negmaskT_cores.append(np.ascontiguousarray(negmask[sl].T[perm, :]))

    _consts = (pri, cand_idx, negmask, perms, negmaskT_cores)
    return _consts


def _select_samples(labels_flat, pri, cand_idx):
    """Exact replication of the reference's per-class sampling.

    Returns (sampled_idx [KMAX] int32) or None if the candidate-filter
    safety conditions fail (caller then uses the exact full fallback).
    """
    cl = labels_flat[cand_idx]
    ck = (cl.astype(np.float32) * np.float32(2.0) + pri[cand_idx]).astype(
        np.float32
    )
    order = np.lexsort((cand_idx, ck))  # == stable argsort of reference key
    cs = cl[order]
    ci = cand_idx[order]
    counts = np.bincount(cs, minlength=C)
    if counts.min() < NUM_SAMPLES:
        return None
    start = np.concatenate(([0], np.cumsum(counts)[:-1]))
    rank = np.arange(cs.size) - start[cs]
    sel = rank < NUM_SAMPLES
    sampled = np.zeros(KMAX, np.int32)
    sampled[cs[sel] * NUM_SAMPLES + rank[sel]] = ci[sel]
    # 64th-smallest priority per class must clear the threshold with margin
    # so no non-candidate could tie/outrank under f32 key rounding.
    p64 = pri[sampled[np.arange(KMAX) % NUM_SAMPLES == NUM_SAMPLES - 1]]
    if p64.max() >= CAND_T_SAFE:
        return None
    return sampled


def _host_fallback(voxel_emb, labels_flat, label_emb, pri):
    """Bit-faithful full replication of the reference (never expected to run)."""
    sort_key = labels_flat.astype(np.float32) * np.float32(2.0) + pri
    sorted_indices = np.argsort(sort_key, kind="stable").astype(np.int32)
    sorted_labels = labels_flat[sorted_indices]
    first_occ = np.full(C, N, np.int64)
    np.minimum.at(first_occ, sorted_labels, np.arange(N))
    positions = np.arange(N) - first_occ[sorted_labels]
    mask = positions < NUM_SAMPLES
    slot = np.where(mask, sorted_labels * NUM_SAMPLES + positions, KMAX)
    sampled = np.zeros(KMAX + 1, np.int32)
    sampled[slot] = sorted_indices
    sampled = sampled[:KMAX]
    valid = np.zeros(KMAX + 1, bool)
    valid[slot] = True
    valid = valid[:KMAX]

    _, _, negmask, _, _ = _build_constants()
    bb = sampled // HWZ
    rr = sampled % HWZ
    anchors = voxel_emb.reshape(B, D, HWZ)[bb, :, rr].astype(np.float32)
    ta = np.sqrt(1.0 + (anchors * anchors).sum(-1, dtype=np.float32)).astype(
        np.float32
    )
    tl = np.sqrt(
        1.0 + (label_emb * label_emb).sum(-1, dtype=np.float32)
    ).astype(np.float32)
    inner = (anchors @ label_emb.T).astype(np.float32) - ta[:, None] * tl[None, :]
    x = np.maximum(-inner, np.float32(1.0 + EPS)).astype(np.float32)
    dmat = np.log(
        x + np.sqrt(x * x - 1.0, dtype=np.float32), dtype=np.float32
    )
    sc = (np.arange(KMAX) // NUM_SAMPLES).astype(np.int32)
    dpos = dmat[np.arange(KMAX), sc]
    tri = np.maximum((dpos[:, None] + np.float32(MARGIN)) - dmat, 0.0) * negmask
    tri *= valid[:, None].astype(np.float32)
    denom = max(float(valid.sum()) * NUM_NEG, 1.0)
    return np.float32(tri.sum(dtype=np.float64) / denom)


# --------------------------------------------------------------------------
# device kernel
# --------------------------------------------------------------------------
def _build_bass():
    global _nc
    if _nc is not None:
        return _nc
    import concourse.bass as bass
    import concourse.tile as tile
    from concourse import bacc, mybir
    from concourse.tile_rust import add_dep_helper

    F = mybir.ActivationFunctionType
    A = mybir.AluOpType
    f32 = mybir.dt.float32

    NW = 2              # two 512-wide chunks per core
    WID = KPC // NW     # 512

    nc = bacc.Bacc("TRN2", target_bir_lowering=False, debug=False)
    aT = nc.dram_tensor("extAT", [D + 1, KPC], f32, kind="ExternalInput").ap()
    lT = nc.dram_tensor("extLT", [D + 1, C], f32, kind="ExternalInput").ap()
    nm = nc.dram_tensor("negmaskT", [C, KPC], f32, kind="ExternalInput").ap()
    dp = nc.dram_tensor("dposm", [1, KPC], f32, kind="ExternalInput").ap()
    out = nc.dram_tensor("partial", [1, 1], f32, kind="ExternalOutput").ap()

    with tile.TileContext(nc) as tc:
        with (
            tc.tile_pool(name="cst", bufs=1) as cst,
            tc.tile_pool(name="sb", bufs=2) as sb,
            tc.tile_pool(name="ps", bufs=2, space="PSUM") as ps,
        ):
            negone = cst.tile([128, 1], f32)
            nc.vector.memset(negone[:], -1.0)
            ones128 = cst.tile([128, 1], f32)
            nc.vector.memset(ones128[:], 1.0)
            scratch = cst.tile([1, 1], f32)

            # preload the Sqrt activation table during the DMA wait window
            d_sq = nc.scalar.activation(scratch[:], negone[0:1, 0:1], F.Sqrt, scale=-1.0)

            # extL/extA carry embeddings (d on partitions) + a 65th row with
            # the (host-computed) Lorentz time component:
            # inner_L = sum_d x_d y_d - t(x) t(y)
            extL = cst.tile([D + 1, C], f32)
            nc.gpsimd.dma_start(out=extL[:, :], in_=lT[:, :])
            extA = cst.tile([D + 1, KPC], f32)
            QW = KPC // 4
            for q in range(4):
                qs = bass.ts(q, QW)
                nc.sync.dma_start(out=extA[:, qs], in_=aT[:, qs])
            dpt = cst.tile([1, KPC], f32)
            nc.gpsimd.dma_start(out=dpt[:, :], in_=dp[:, :])
            nmT = cst.tile([C, KPC], f32)
            for q in range(4):
                qs = bass.ts(q, KPC // 4)
                nc.gpsimd.dma_start(out=nmT[:, qs], in_=nm[:, qs])

            qcol = cst.tile([C, NW], f32)
            sqrt_insts = [d_sq]
            ln_insts = []

            for j in range(NW):
                cols = bass.ts(j, WID)

                # broadcast margin+positive-distance row across partitions on
                # gpsimd; input-only dependency, so this runs early
                dpmb = sb.tile([C, WID], f32)
                nc.gpsimd.partition_broadcast(dpmb[:], dpt[0:1, cols])

                ps_in = ps.tile([C, WID], f32)
                nc.tensor.matmul(
                    ps_in[:], lhsT=extL[:, :], rhs=extA[:, cols], start=True, stop=True
                )

                # x = max(-inner, 1+eps); d = acosh(x) = ln(x + sqrt(x^2-1))
                xt = sb.tile([C, WID], f32)
                nc.vector.tensor_scalar(
                    xt[:], ps_in[:], -1.0, 1.0 + EPS, op0=A.mult, op1=A.max
                )
                t1 = sb.tile([C, WID], f32)
                sqrt_insts.append(
                    nc.scalar.activation(t1[:], xt[:], F.Square)
                )
                st = sb.tile([C, WID], f32)
                sqrt_insts.append(
                    nc.scalar.activation(st[:], t1[:], F.Sqrt, bias=negone[:])
                )
                nc.vector.tensor_tensor(st[:], st[:], xt[:], op=A.add)
                dmat = sb.tile([C, WID], f32)
                ln_insts.append(nc.scalar.activation(dmat[:], st[:], F.Ln))

                ut = sb.tile([C, WID], f32)
                nc.vector.tensor_tensor(ut[:], dpmb[:], dmat[:], op=A.subtract)
                vt = sb.tile([C, WID], f32)
                nc.vector.scalar_tensor_tensor(
                    out=vt[:],
                    in0=ut[:],
                    scalar=0.0,
                    in1=nmT[:, cols],
                    op0=A.max,
                    op1=A.mult,
                    accum_out=qcol[:, j : j + 1],
                )

            # preload the Ln table in the gap between the sqrt and ln phases
            d_ln = nc.scalar.activation(scratch[:], negone[0:1, 0:1], F.Ln, scale=-1.0)
            for s_i in sqrt_insts:
                add_dep_helper(d_ln.ins, s_i.ins, False, "act-table order")
            ln_insts.append(d_ln)

            # keep the scalar engine's activation table from thrashing:
            # all Sqrt ops strictly before any Ln op
            for s_i in sqrt_insts:
                for l_i in ln_insts:
                    if l_i is d_ln:
                        continue
                    add_dep_helper(l_i.ins, s_i.ins, False, "act-table order")
                    add_dep_helper(l_i.ins, d_ln.ins, False, "act-table order")

            rs1 = cst.tile([C, 1], f32)
            nc.vector.tensor_reduce(
                rs1[:], qcol[:], axis=mybir.AxisListType.X, op=A.add
            )
            ps_s = ps.tile([1, 1], f32, bufs=1)
            nc.tensor.matmul(ps_s[:], lhsT=ones128[:], rhs=rs1[:], start=True, stop=True)
            outt = cst.tile([1, 1], f32)
            nc.vector.tensor_copy(outt[:], ps_s[:])
            nc.scalar.dma_start(out=out[:, :], in_=outt[:])

    nc.compile()
    _nc = nc
    return nc


# --------------------------------------------------------------------------
# entry point
# --------------------------------------------------------------------------
def kernel(voxel_emb, labels, label_emb, _run_kwargs=None):
    from concourse.bass_utils import run_bass_kernel_spmd

    voxel_emb = np.asarray(voxel_emb)
    label_emb = np.ascontiguousarray(np.asarray(label_emb), dtype=np.float32)
    labels_flat = (
        np.asarray(labels).reshape(-1).astype(np.int32, copy=False)
    )

    pri, cand_idx, negmask, perms, negmaskT_cores = _build_constants()

    sampled = _select_samples(labels_flat, pri, cand_idx)
    if sampled is None:  # astronomically unlikely; exact host fallback
        return _host_fallback(
            np.asarray(voxel_emb, dtype=np.float32), labels_flat, label_emb, pri
        )

    # host gather of the 8192 selected anchor rows (strided in voxel_emb)
    bb = sampled // HWZ
    rr = sampled % HWZ
    anchors = voxel_emb.reshape(B, D, HWZ)[bb, :, rr].astype(
        np.float32, copy=False
    )  # [KMAX, D]

    # host-computed Lorentz time components appended as row 64
    t_a = np.sqrt(1.0 + (anchors * anchors).sum(1, dtype=np.float32)).astype(
        np.float32
    )  # [KMAX]
    t_l = np.sqrt(
        1.0 + (label_emb * label_emb).sum(1, dtype=np.float32)
    ).astype(np.float32)  # [C]

    # host-computed positive (pointwise) distances + margin: O(K*D) work
    sc = (np.arange(KMAX) // NUM_SAMPLES).astype(np.int32)
    pos = label_emb[sc]  # [KMAX, D]
    inner_p = (
        (anchors * pos).sum(1, dtype=np.float32) - t_a * t_l[sc]
    ).astype(np.float32)
    xp = np.maximum(-inner_p, np.float32(1.0 + EPS))
    dposm = (
        np.log(xp + np.sqrt(xp * xp - 1.0, dtype=np.float32), dtype=np.float32)
        + np.float32(MARGIN)
    ).astype(np.float32)  # [KMAX]

    labelT = label_emb.T  # [D, C]
    nc = _build_bass()
    in_maps = []
    for i in range(NCORES):
        sl = slice(i * KPC, (i + 1) * KPC)
        extAT = np.empty((D + 1, KPC), np.float32)
        extAT[0:D] = anchors[sl].T
        extAT[D] = t_a[sl]
        extLT = np.empty((D + 1, C), np.float32)
        extLT[0:D] = labelT[:, perms[i]]
        extLT[D] = -t_l[perms[i]]
        in_maps.append(
            {
                "extAT": extAT,
                "extLT": extLT,
                "negmaskT": negmaskT_cores[i],
                "dposm": np.ascontiguousarray(dposm[None, sl]),
            }
        )
    res = run_bass_kernel_spmd(
        nc, in_maps, core_ids=list(range(NCORES)), **(_run_kwargs or {})
    )
    total = sum(float(r["partial"][0, 0]) for r in res.results)
    loss = np.float32(total / float(KMAX * NUM_NEG))
    if _run_kwargs:
        kernel.last_results = res
    return np.array(loss, dtype=np.float32)


# revision 19
# speedup vs baseline: 1.1231x; 1.1231x over previous
"""Trainium2 Bass kernel for nn_LorentzRankingLoss.

Contract: kernel(**inputs) takes FULL unsharded numpy inputs
(voxel_emb [2,64,96,96,96] f32, labels [2,96,96,96] int, label_emb [128,64] f32)
and returns the FULL output (scalar f32 loss), distributing work over 8
NeuronCores internally.

Algorithm notes
---------------
The reference samples NUM_SAMPLES=64 voxels per class (128 classes) by a
stable argsort of key = label*2 + pri where pri = uniform(jax key 42) is an
*input-independent* constant.  Likewise the random negative-class choices
per sampled slot are input-independent.  So:

* pri, the candidate set {n : pri[n] < T}, and the negative-selection masks
  are compile-time constants (computed once, host side).
* The top-64-per-class selection only needs the labels of the ~17.6k
  candidate voxels (a class's 64 smallest priorities all lie below T=0.01
  with astronomically high probability; verified at runtime with an exact
  full fallback).
* The 8192 selected anchor rows are gathered on host (strided rows of
  voxel_emb) and the 8 NeuronCores compute the Lorentz distance matrix
  (8192x128 via TensorE with a +time extended contraction), acosh, the
  margin/negative-masked triplet terms and the full reduction; each core
  handles 1024 slots and emits one partial sum.
"""

import numpy as np

# ---- problem constants (hardcoded per spec) ----
NUM_SAMPLES = 64
NUM_NEG = 8
C = 128
MARGIN = 0.1
CURV = 1.0
EPS = 1e-7
B, D, H, W, Z = 2, 64, 96, 96, 96
HWZ = H * W * Z
N = B * HWZ                      # 1_769_472
KMAX = C * NUM_SAMPLES           # 8192
NCORES = 8
KPC = KMAX // NCORES             # 1024 slots per core
NCHUNK = KPC // 128              # 8 chunks of 128 anchors
CAND_T = np.float32(0.01)        # candidate priority threshold
CAND_T_SAFE = np.float32(0.01 - 1e-4)

_consts = None                   # lazy: (pri, cand_idx, negmask, posmask)
_nc = None                       # lazy: compiled bass program


# --------------------------------------------------------------------------
# host-side constants (input independent)
# --------------------------------------------------------------------------
def _build_constants():
    global _consts
    if _consts is not None:
        return _consts
    import jax
    import jax.numpy as jnp

    cpu = jax.devices("cpu")[0]
    with jax.default_device(cpu):
        key = jax.random.key(42)
        k_pri, k_neg = jax.random.split(key)
        pri = np.asarray(jax.random.uniform(k_pri, (N,), dtype=jnp.float32))
        neg_scores = np.asarray(
            jax.random.uniform(k_neg, (KMAX, C), dtype=jnp.float32)
        )

    cand_idx = np.nonzero(pri < CAND_T)[0].astype(np.int32)

    sampled_classes = (np.arange(KMAX) // NUM_SAMPLES).astype(np.int32)
    nmask_bool = np.arange(C)[None, :] != sampled_classes[:, None]
    scores = np.where(nmask_bool, neg_scores, -1.0).astype(np.float32)
    # jax.lax.top_k: descending, ties -> lower index first == stable argsort
    neg_idx = np.argsort(-scores, axis=1, kind="stable")[:, :NUM_NEG]
    negmask = np.zeros((KMAX, C), np.float32)
    np.put_along_axis(negmask, neg_idx, 1.0, axis=1)

    # per-core class permutation: the 16 classes whose slots a core owns
    # come first, so positive-distance extraction is SPMD-uniform
    perms = []
    negmaskT_cores = []
    for i in range(NCORES):
        own = np.arange(16 * i, 16 * (i + 1))
        rest = np.concatenate([np.arange(0, 16 * i), np.arange(16 * (i + 1), C)])
        perm = np.concatenate([own, rest]).astype(np.int64)
        perms.append(perm)
        sl = slice(i * KPC, (i + 1) * KPC)
        negmaskT_cores.append(np.ascontiguousarray(negmask[sl].T[perm, :]))

    _consts = (pri, cand_idx, negmask, perms, negmaskT_cores)
    return _consts


def _select_samples(labels_flat, pri, cand_idx):
    """Exact replication of the reference's per-class sampling.

    Returns (sampled_idx [KMAX] int32) or None if the candidate-filter
    safety conditions fail (caller then uses the exact full fallback).
    """
    cl = labels_flat[cand_idx]
    ck = (cl.astype(np.float32) * np.float32(2.0) + pri[cand_idx]).astype(
        np.float32
    )
    order = np.lexsort((cand_idx, ck))  # == stable argsort of reference key
    cs = cl[order]
    ci = cand_idx[order]
    counts = np.bincount(cs, minlength=C)
    if counts.min() < NUM_SAMPLES:
        return None
    start = np.concatenate(([0], np.cumsum(counts)[:-1]))
    rank = np.arange(cs.size) - start[cs]
    sel = rank < NUM_SAMPLES
    sampled = np.zeros(KMAX, np.int32)
    sampled[cs[sel] * NUM_SAMPLES + rank[sel]] = ci[sel]
    # 64th-smallest priority per class must clear the threshold with margin
    # so no non-candidate could tie/outrank under f32 key rounding.
    p64 = pri[sampled[np.arange(KMAX) % NUM_SAMPLES == NUM_SAMPLES - 1]]
    if p64.max() >= CAND_T_SAFE:
        return None
    return sampled


def _host_fallback(voxel_emb, labels_flat, label_emb, pri):
    """Bit-faithful full replication of the reference (never expected to run)."""
    sort_key = labels_flat.astype(np.float32) * np.float32(2.0) + pri
    sorted_indices = np.argsort(sort_key, kind="stable").astype(np.int32)
    sorted_labels = labels_flat[sorted_indices]
    first_occ = np.full(C, N, np.int64)
    np.minimum.at(first_occ, sorted_labels, np.arange(N))
    positions = np.arange(N) - first_occ[sorted_labels]
    mask = positions < NUM_SAMPLES
    slot = np.where(mask, sorted_labels * NUM_SAMPLES + positions, KMAX)
    sampled = np.zeros(KMAX + 1, np.int32)
    sampled[slot] = sorted_indices
    sampled = sampled[:KMAX]
    valid = np.zeros(KMAX + 1, bool)
    valid[slot] = True
    valid = valid[:KMAX]

    _, _, negmask, _, _ = _build_constants()
    bb = sampled // HWZ
    rr = sampled % HWZ
    anchors = voxel_emb.reshape(B, D, HWZ)[bb, :, rr].astype(np.float32)
    ta = np.sqrt(1.0 + (anchors * anchors).sum(-1, dtype=np.float32)).astype(
        np.float32
    )
    tl = np.sqrt(
        1.0 + (label_emb * label_emb).sum(-1, dtype=np.float32)
    ).astype(np.float32)
    inner = (anchors @ label_emb.T).astype(np.float32) - ta[:, None] * tl[None, :]
    x = np.maximum(-inner, np.float32(1.0 + EPS)).astype(np.float32)
    dmat = np.log(
        x + np.sqrt(x * x - 1.0, dtype=np.float32), dtype=np.float32
    )
    sc = (np.arange(KMAX) // NUM_SAMPLES).astype(np.int32)
    dpos = dmat[np.arange(KMAX), sc]
    tri = np.maximum((dpos[:, None] + np.float32(MARGIN)) - dmat, 0.0) * negmask
    tri *= valid[:, None].astype(np.float32)
    denom = max(float(valid.sum()) * NUM_NEG, 1.0)
    return np.float32(tri.sum(dtype=np.float64) / denom)


# --------------------------------------------------------------------------
# device kernel
# --------------------------------------------------------------------------
def _build_bass():
    global _nc
    if _nc is not None:
        return _nc
    import concourse.bass as bass
    import concourse.tile as tile
    from concourse import bacc, mybir
    from concourse.tile_rust import add_dep_helper

    F = mybir.ActivationFunctionType
    A = mybir.AluOpType
    f32 = mybir.dt.float32

    NW = 2              # two 512-wide chunks per core
    WID = KPC // NW     # 512

    nc = bacc.Bacc("TRN2", target_bir_lowering=False, debug=False)
    aT = nc.dram_tensor("extAT", [D + 1, KPC], f32, kind="ExternalInput").ap()
    lT = nc.dram_tensor("extLT", [D + 1, C], f32, kind="ExternalInput").ap()
    nm = nc.dram_tensor("negmaskT", [C, KPC], f32, kind="ExternalInput").ap()
    dp = nc.dram_tensor("dposm", [1, KPC], f32, kind="ExternalInput").ap()
    out = nc.dram_tensor("partial", [1, 1], f32, kind="ExternalOutput").ap()

    with tile.TileContext(nc) as tc:
        with (
            tc.tile_pool(name="cst", bufs=1) as cst,
            tc.tile_pool(name="sb", bufs=2) as sb,
            tc.tile_pool(name="ps", bufs=2, space="PSUM") as ps,
        ):
            negone = cst.tile([128, 1], f32)
            nc.vector.memset(negone[:], -1.0)
            ones128 = cst.tile([128, 1], f32)
            nc.vector.memset(ones128[:], 1.0)
            scratch = cst.tile([1, 1], f32)

            # preload the Sqrt activation table during the DMA wait window
            d_sq = nc.scalar.activation(scratch[:], negone[0:1, 0:1], F.Sqrt, scale=-1.0)

            # extL/extA carry embeddings (d on partitions) + a 65th row with
            # the (host-computed) Lorentz time component:
            # inner_L = sum_d x_d y_d - t(x) t(y)
            extL = cst.tile([D + 1, C], f32)
            nc.gpsimd.dma_start(out=extL[:, :], in_=lT[:, :])
            extA = cst.tile([D + 1, KPC], f32)
            QW = KPC // 4
            for q in range(4):
                qs = bass.ts(q, QW)
                nc.sync.dma_start(out=extA[:, qs], in_=aT[:, qs])
            dpt = cst.tile([1, KPC], f32)
            nc.gpsimd.dma_start(out=dpt[:, :], in_=dp[:, :])
            nmT = cst.tile([C, KPC], f32)
            for q in range(4):
                qs = bass.ts(q, KPC // 4)
                nc.gpsimd.dma_start(out=nmT[:, qs], in_=nm[:, qs])

            qcol = cst.tile([C, NW], f32)
            sqrt_insts = [d_sq]
            ln_insts = []

            for j in range(NW):
                cols = bass.ts(j, WID)

                # broadcast margin+positive-distance row across partitions on
                # gpsimd; input-only dependency, so this runs early
                dpmb = sb.tile([C, WID], f32)
                nc.gpsimd.partition_broadcast(dpmb[:], dpt[0:1, cols])

                ps_in = ps.tile([C, WID], f32)
                nc.tensor.matmul(
                    ps_in[:], lhsT=extL[:, :], rhs=extA[:, cols], start=True, stop=True
                )

                # x = max(-inner, 1+eps); d = acosh(x) = ln(x + sqrt(x^2-1))
                xt = sb.tile([C, WID], f32)
                nc.vector.tensor_scalar(
                    xt[:], ps_in[:], -1.0, 1.0 + EPS, op0=A.mult, op1=A.max
                )
                t1 = sb.tile([C, WID], f32)
                nc.vector.tensor_tensor(t1[:], xt[:], xt[:], op=A.mult)
                st = sb.tile([C, WID], f32)
                sqrt_insts.append(
                    nc.scalar.activation(st[:], t1[:], F.Sqrt, bias=negone[:])
                )
                nc.vector.tensor_tensor(st[:], st[:], xt[:], op=A.add)
                dmat = sb.tile([C, WID], f32)
                ln_insts.append(nc.scalar.activation(dmat[:], st[:], F.Ln))

                ut = sb.tile([C, WID], f32)
                nc.vector.tensor_tensor(ut[:], dpmb[:], dmat[:], op=A.subtract)
                vt = sb.tile([C, WID], f32)
                nc.vector.scalar_tensor_tensor(
                    out=vt[:],
                    in0=ut[:],
                    scalar=0.0,
                    in1=nmT[:, cols],
                    op0=A.max,
                    op1=A.mult,
                    accum_out=qcol[:, j : j + 1],
                )

            # preload the Ln table in the gap between the sqrt and ln phases
            d_ln = nc.scalar.activation(scratch[:], negone[0:1, 0:1], F.Ln, scale=-1.0)
            for s_i in sqrt_insts:
                add_dep_helper(d_ln.ins, s_i.ins, False, "act-table order")
            ln_insts.append(d_ln)

            # keep the scalar engine's activation table from thrashing:
            # all Sqrt ops strictly before any Ln op
            for s_i in sqrt_insts:
                for l_i in ln_insts:
                    if l_i is d_ln:
                        continue
                    add_dep_helper(l_i.ins, s_i.ins, False, "act-table order")
                    add_dep_helper(l_i.ins, d_ln.ins, False, "act-table order")

            rs1 = cst.tile([C, 1], f32)
            nc.vector.tensor_reduce(
                rs1[:], qcol[:], axis=mybir.AxisListType.X, op=A.add
            )
            ps_s = ps.tile([1, 1], f32, bufs=1)
            nc.tensor.matmul(ps_s[:], lhsT=ones128[:], rhs=rs1[:], start=True, stop=True)
            outt = cst.tile([1, 1], f32)
            nc.vector.tensor_copy(outt[:], ps_s[:])
            nc.scalar.dma_start(out=out[:, :], in_=outt[:])

    nc.compile()
    _nc = nc
    return nc


# --------------------------------------------------------------------------
# entry point
# --------------------------------------------------------------------------
def kernel(voxel_emb, labels, label_emb, _run_kwargs=None):
    from concourse.bass_utils import run_bass_kernel_spmd

    voxel_emb = np.asarray(voxel_emb)
    label_emb = np.ascontiguousarray(np.asarray(label_emb), dtype=np.float32)
    labels_flat = (
        np.asarray(labels).reshape(-1).astype(np.int32, copy=False)
    )

    pri, cand_idx, negmask, perms, negmaskT_cores = _build_constants()

    sampled = _select_samples(labels_flat, pri, cand_idx)
    if sampled is None:  # astronomically unlikely; exact host fallback
        return _host_fallback(
            np.asarray(voxel_emb, dtype=np.float32), labels_flat, label_emb, pri
        )

    # host gather of the 8192 selected anchor rows (strided in voxel_emb)
    bb = sampled // HWZ
    rr = sampled % HWZ
    anchors = voxel_emb.reshape(B, D, HWZ)[bb, :, rr].astype(
        np.float32, copy=False
    )  # [KMAX, D]

    # host-computed Lorentz time components appended as row 64
    t_a = np.sqrt(1.0 + (anchors * anchors).sum(1, dtype=np.float32)).astype(
        np.float32
    )  # [KMAX]
    t_l = np.sqrt(
        1.0 + (label_emb * label_emb).sum(1, dtype=np.float32)
    ).astype(np.float32)  # [C]

    # host-computed positive (pointwise) distances + margin: O(K*D) work
    sc = (np.arange(KMAX) // NUM_SAMPLES).astype(np.int32)
    pos = label_emb[sc]  # [KMAX, D]
    inner_p = (
        (anchors * pos).sum(1, dtype=np.float32) - t_a * t_l[sc]
    ).astype(np.float32)
    xp = np.maximum(-inner_p, np.float32(1.0 + EPS))
    dposm = (
        np.log(xp + np.sqrt(xp * xp - 1.0, dtype=np.float32), dtype=np.float32)
        + np.float32(MARGIN)
    ).astype(np.float32)  # [KMAX]

    labelT = label_emb.T  # [D, C]
    nc = _build_bass()
    in_maps = []
    for i in range(NCORES):
        sl = slice(i * KPC, (i + 1) * KPC)
        extAT = np.empty((D + 1, KPC), np.float32)
        extAT[0:D] = anchors[sl].T
        extAT[D] = t_a[sl]
        extLT = np.empty((D + 1, C), np.float32)
        extLT[0:D] = labelT[:, perms[i]]
        extLT[D] = -t_l[perms[i]]
        in_maps.append(
            {
                "extAT": extAT,
                "extLT": extLT,
                "negmaskT": negmaskT_cores[i],
                "dposm": np.ascontiguousarray(dposm[None, sl]),
            }
        )
    res = run_bass_kernel_spmd(
        nc, in_maps, core_ids=list(range(NCORES)), **(_run_kwargs or {})
    )
    total = sum(float(r["partial"][0, 0]) for r in res.results)
    loss = np.float32(total / float(KMAX * NUM_NEG))
    if _run_kwargs:
        kernel.last_results = res
    return np.array(loss, dtype=np.float32)
